# revision 6
# baseline (speedup 1.0000x reference)
"""MGF (multi-scale gradient features) Trainium2 kernel.

Full inputs -> full outputs; internally sharded batch-parallel over 8 cores.

Math (per scale k, K=2k+1, xp = reflect_pad(x, k) + 0.01):
  g0 = Vfull * Hleft   g1 = Vfull * Hright   (gx = ln g0 - ln g1)
  g2 = Vtop * Hfull    g3 = Vbot * Hfull     (gy = ln g2 - ln g3)
  out = sqrt(gx^2 + gy^2)

Device algorithm per (image, scale), in [row, col] layout:
  CK  = vertical box (width K)   of xp  -> banded matmul on TensorE
  Ck1 = vertical box (width k+1) of xp  -> banded matmul on TensorE
  S2  = horizontal box (width k+1) of CK  -> tensor_tensor_scan on VectorE
  D   = horizontal box (width K)   of Ck1 -> tensor_tensor_scan on VectorE
  gx[y][x] = lnS2[y][x]   - lnS2[y][x+k]      (free-dim shift)
  gy[y][x] = lnD_arr[y][x] - lnD_arr[y+k][x]  (partition shift via DMA realign)
  out = sqrt(gx^2 + gy^2)
"""
import sys
if '/opt/trn_rl_repo' not in sys.path:
    sys.path.insert(0, '/opt/trn_rl_repo')
import numpy as np

KS = (9, 13, 17)
PAD = 18          # host reflect pad; >= max(k)+1
N_CORES = 8
IMG = 4           # images per core
H = W = 512
HP = H + 2 * PAD  # 548
ROWS = [128, 128, 128, 128, HP - 512]  # stored xp row tiles (last = 36)


def _make_weights():
    """Banded ones matrices, tile-invariant under the aligned storage scheme.

    Stored xp tile t partition p  <->  xp coordinate row r = 128t + p - PAD.
    CK  out-tile t partition o    <->  y  = 128t + o          (band |r-y| <= k)
    Ck1 out-tile t partition o    <->  y' = 128t + o - k      (band r-y' in [0,k])
    W0*: contraction against stored tile t     (r - y = p - PAD - o)
    W1*: contraction against stored tile t+1   (r - y = p + 128 - PAD - o)
    """
    p = np.arange(128)[:, None]
    o = np.arange(128)[None, :]
    d0 = p - PAD - o
    d1 = p + 128 - PAD - o
    ws = []
    for k in KS:
        W0K = ((d0 >= -k) & (d0 <= k)).astype(np.float16)
        W1K = ((d1 >= -k) & (d1 <= k)).astype(np.float16)
        W0k1 = ((d0 >= -k) & (d0 <= 0)).astype(np.float16)
        W1k1 = ((d1 >= -k) & (d1 <= 0)).astype(np.float16)
        ws.append(np.stack([W0K, W1K, W0k1, W1k1]))
    return np.stack(ws)  # [3, 4, 128, 128] fp16


_CACHE = {}


def build(trace_sim=False):
    """Build + compile the per-core Bass kernel. Returns the Bacc module."""
    if 'nc' in _CACHE:
        return _CACHE['nc']
    import concourse.bacc as bacc
    import concourse.tile as tile
    from concourse import mybir
    from contextlib import ExitStack

    f32 = mybir.dt.float32
    f16 = mybir.dt.float16
    ADD = mybir.AluOpType.add
    SUB = mybir.AluOpType.subtract
    AXX = mybir.AxisListType.X
    LN = mybir.ActivationFunctionType.Ln
    EXP = mybir.ActivationFunctionType.Exp

    nc = bacc.Bacc("TRN2", target_bir_lowering=False, debug=False,
                   num_devices=N_CORES)
    xp_d = nc.dram_tensor("xp", [IMG, HP, HP], f16, kind="ExternalInput").ap()
    w_d = nc.dram_tensor("wts", [3, 4, 128, 128], f16, kind="ExternalInput").ap()
    y_d = nc.dram_tensor("y", [IMG, 3, H, W], f32, kind="ExternalOutput").ap()

    with tile.TileContext(nc, trace_sim=trace_sim) as tc, ExitStack() as ctx:
        wp = ctx.enter_context(tc.tile_pool(name="wp", bufs=12))
        xpool = ctx.enter_context(tc.tile_pool(name="xpool", bufs=10))
        pspool = ctx.enter_context(
            tc.tile_pool(name="ps", bufs=8, space="PSUM"))
        ckp = ctx.enter_context(tc.tile_pool(name="ckp", bufs=6))
        c1p = ctx.enter_context(tc.tile_pool(name="c1p", bufs=7))
        s2p = ctx.enter_context(tc.tile_pool(name="s2p", bufs=6))
        ddp = ctx.enter_context(tc.tile_pool(name="ddp", bufs=7))
        gxp = ctx.enter_context(tc.tile_pool(name="gxp", bufs=6))
        gyp = ctx.enter_context(tc.tile_pool(name="gyp", bufs=4))
        shp = ctx.enter_context(tc.tile_pool(name="shp", bufs=4))
        resp = ctx.enter_context(tc.tile_pool(name="resp", bufs=4))
        inip = ctx.enter_context(tc.tile_pool(name="inip", bufs=16))

        # stationary weights, loaded once
        wt = {}
        for s in range(3):
            for j in range(4):
                t = wp.tile([128, 128], f16, tag="w")
                nc.sync.dma_start(t[:], w_d[s, j])
                wt[(s, j)] = t

        for img in range(IMG):
            # load padded image rows into 5 aligned tiles
            xt = []
            for m in range(5):
                t = xpool.tile([ROWS[m], HP], f16, tag="xp")
                nc.sync.dma_start(t[:], xp_d[img, 128 * m: 128 * m + ROWS[m], :])
                xt.append(t)

            for s, k in enumerate(KS):
                c0 = PAD - k - 1            # xp col index of v = -k-1
                ncol = 513 + 2 * k          # CK/Ck1 cols: v in [-k-1, 511+k]
                nb = ncol - 512             # second psum chunk (2k+1)
                wb = PAD + k                # rows of W1 slice that matter (<=35)

                # ---------------- gx chain: CK = vbox_K(xp) ----------------
                gx_tiles = []
                for t in range(4):
                    psA = pspool.tile([128, 512], f32, tag="ps")
                    psB = pspool.tile([128, nb], f32, tag="ps")
                    w0, w1 = wt[(s, 0)], wt[(s, 1)]
                    nc.tensor.matmul(psA[:], w0[:], xt[t][:, c0:c0 + 512],
                                     start=True, stop=False)
                    nc.tensor.matmul(psB[:], w0[:], xt[t][:, c0 + 512:c0 + ncol],
                                     start=True, stop=False)
                    nc.tensor.matmul(psA[:], w1[0:wb, :],
                                     xt[t + 1][0:wb, c0:c0 + 512],
                                     start=False, stop=True)
                    nc.tensor.matmul(psB[:], w1[0:wb, :],
                                     xt[t + 1][0:wb, c0 + 512:c0 + ncol],
                                     start=False, stop=True)
                    cks = ckp.tile([128, ncol], f32, tag="ck")
                    nc.scalar.copy(cks[:, 0:512], psA[:])
                    nc.scalar.copy(cks[:, 512:ncol], psB[:])
                    # S2[y][x'] = sum_{v=x'-k..x'} CK[y][v],  x' in [0, 511+k]
                    ini = inip.tile([128, 1], f32, tag="ini")
                    nc.vector.reduce_sum(ini[:], cks[:, 0:k + 1], axis=AXX)
                    s2 = s2p.tile([128, 512 + k], f32, tag="s2")
                    nc.vector.tensor_tensor_scan(
                        s2[:], cks[:, k + 1:k + 1 + 512 + k],
                        cks[:, 0:512 + k], ini[:], ADD, SUB)
                    nc.scalar.activation(s2[:], s2[:], LN)
                    gx = gxp.tile([128, 512], f32, tag="gx")
                    nc.vector.tensor_sub(gx[:], s2[:, 0:512], s2[:, k:k + 512])
                    gx_tiles.append(gx)

                # ---------------- gy chain: Ck1 = vbox_{k+1}(xp) -----------
                # out array row j = y' + k in [0, 528]; 5 tiles (last 17 rows)
                lnd = []
                for t in range(5):
                    ro = 128 if t < 4 else 17
                    psA = pspool.tile([128, 512], f32, tag="ps")
                    psB = pspool.tile([128, nb], f32, tag="ps")
                    w2, w3 = wt[(s, 2)], wt[(s, 3)]
                    if t < 4:
                        nc.tensor.matmul(psA[:], w2[:], xt[t][:, c0:c0 + 512],
                                         start=True, stop=False)
                        nc.tensor.matmul(psB[:], w2[:],
                                         xt[t][:, c0 + 512:c0 + ncol],
                                         start=True, stop=False)
                        nc.tensor.matmul(psA[:], w3[0:wb, :],
                                         xt[t + 1][0:wb, c0:c0 + 512],
                                         start=False, stop=True)
                        nc.tensor.matmul(psB[:], w3[0:wb, :],
                                         xt[t + 1][0:wb, c0 + 512:c0 + ncol],
                                         start=False, stop=True)
                    else:
                        nc.tensor.matmul(psA[0:ro, :], w2[0:ROWS[4], 0:ro],
                                         xt[4][:, c0:c0 + 512],
                                         start=True, stop=True)
                        nc.tensor.matmul(psB[0:ro, :], w2[0:ROWS[4], 0:ro],
                                         xt[4][:, c0 + 512:c0 + ncol],
                                         start=True, stop=True)
                    c1 = c1p.tile([128, ncol], f32, tag="c1")
                    nc.scalar.copy(c1[0:ro, 0:512], psA[0:ro, :])
                    nc.scalar.copy(c1[0:ro, 512:ncol], psB[0:ro, :])
                    # D[j][x] = sum_{v=x-k..x+k} Ck1[j][v], x in [0, 511]
                    ini = inip.tile([128, 1], f32, tag="ini")
                    nc.vector.reduce_sum(ini[0:ro, :], c1[0:ro, 0:2 * k + 1],
                                         axis=AXX)
                    dd = ddp.tile([128, 512], f32, tag="dd")
                    nc.vector.tensor_tensor_scan(
                        dd[0:ro, :], c1[0:ro, 2 * k + 1:2 * k + 1 + 512],
                        c1[0:ro, 0:512], ini[0:ro, :], ADD, SUB)
                    nc.scalar.activation(dd[0:ro, :], dd[0:ro, :], LN)
                    lnd.append(dd)

                # ------------- combine: gy, square-sum, sqrt, store --------
                for t in range(4):
                    sh = shp.tile([128, 512], f32, tag="sh")
                    nc.sync.dma_start(sh[0:128 - k, :], lnd[t][k:128, :])
                    nc.sync.dma_start(sh[128 - k:128, :], lnd[t + 1][0:k, :])
                    gy = gyp.tile([128, 512], f32, tag="gy")
                    nc.vector.tensor_sub(gy[:], lnd[t][:], sh[:])
                    gx = gx_tiles[t]
                    nc.vector.tensor_mul(gx[:], gx[:], gx[:])
                    nc.vector.tensor_mul(gy[:], gy[:], gy[:])
                    # s = gx^2 + gy^2 + tiny (tiny keeps ln finite when s==0)
                    nc.vector.scalar_tensor_tensor(gx[:], gx[:], 1e-38, gy[:],
                                                   ADD, ADD)
                    # sqrt(s) = exp(0.5*ln(s)): keeps every ACT op in the
                    # natural_log_exp_and_others table set (no ~2.7us swaps)
                    nc.scalar.activation(gx[:], gx[:], LN)
                    res = resp.tile([128, 512], f32, tag="res")
                    nc.scalar.activation(res[:], gx[:], EXP, scale=0.5)
                    nc.sync.dma_start(y_d[img, s, 128 * t:128 * t + 128, :],
                                      res[:])

    nc.compile()
    _CACHE['nc'] = nc
    return nc


def prep_inputs(x):
    """x: [32,1,512,512] f32 -> per-core input maps."""
    x2 = np.asarray(x, dtype=np.float32).reshape(32, 512, 512)
    xp = np.pad(x2, ((0, 0), (PAD, PAD), (PAD, PAD)), mode='reflect') + 0.01
    xp16 = xp.astype(np.float16)
    wts = _make_weights()
    return [{"xp": np.ascontiguousarray(xp16[c * IMG:(c + 1) * IMG]),
             "wts": wts} for c in range(N_CORES)]


def run(x, trace=False, **kw):
    from concourse.bass_utils import run_bass_kernel_spmd
    nc = build()
    in_maps = prep_inputs(x)
    res = run_bass_kernel_spmd(nc, in_maps, list(range(N_CORES)),
                               trace=trace, **kw)
    out = np.concatenate([res.results[c]["y"] for c in range(N_CORES)], axis=0)
    return out.astype(np.float32), res


def kernel(x):
    out, _ = run(x, trace=False)
    return out


# revision 9
# speedup vs baseline: 1.3370x; 1.3370x over previous
"""MGF (multi-scale gradient features) Trainium2 kernel.

Full inputs -> full outputs; internally sharded batch-parallel over 8 cores.

Math (per scale k, K=2k+1, xp = reflect_pad(x, k) + 0.01):
  g0 = Vfull * Hleft   g1 = Vfull * Hright   (gx = ln g0 - ln g1)
  g2 = Vtop * Hfull    g3 = Vbot * Hfull     (gy = ln g2 - ln g3)
  out = sqrt(gx^2 + gy^2)

Device algorithm per (image, scale), in [row, col] layout:
  CK  = vertical box (width K)   of xp  -> banded matmul on TensorE
  Ck1 = vertical box (width k+1) of xp  -> banded matmul on TensorE
  S2  = horizontal box (width k+1) of CK  -> tensor_tensor_scan on VectorE
  D   = horizontal box (width K)   of Ck1 -> tensor_tensor_scan on VectorE
  gx[y][x] = lnS2[y][x]   - lnS2[y][x+k]      (free-dim shift)
  gy[y][x] = lnD_arr[y][x] - lnD_arr[y+k][x]  (partition shift via DMA realign)
  out = sqrt(gx^2 + gy^2)
"""
import sys
if '/opt/trn_rl_repo' not in sys.path:
    sys.path.insert(0, '/opt/trn_rl_repo')
import numpy as np

KS = (9, 13, 17)
PAD = 18          # host reflect pad; >= max(k)+1
N_CORES = 8
IMG = 4           # images per core
H = W = 512
HP = H + 2 * PAD  # 548
ROWS = [128, 128, 128, 128, HP - 512]  # stored xp row tiles (last = 36)


def _make_weights():
    """Banded ones matrices, tile-invariant under the aligned storage scheme.

    Stored xp tile t partition p  <->  xp coordinate row r = 128t + p - PAD.
    CK  out-tile t partition o    <->  y  = 128t + o          (band |r-y| <= k)
    Ck1 out-tile t partition o    <->  y' = 128t + o - k      (band r-y' in [0,k])
    W0*: contraction against stored tile t     (r - y = p - PAD - o)
    W1*: contraction against stored tile t+1   (r - y = p + 128 - PAD - o)
    """
    p = np.arange(128)[:, None]
    o = np.arange(128)[None, :]
    d0 = p - PAD - o
    d1 = p + 128 - PAD - o
    ws = []
    for k in KS:
        W0K = ((d0 >= -k) & (d0 <= k)).astype(np.float16)
        W1K = ((d1 >= -k) & (d1 <= k)).astype(np.float16)
        W0k1 = ((d0 >= -k) & (d0 <= 0)).astype(np.float16)
        W1k1 = ((d1 >= -k) & (d1 <= 0)).astype(np.float16)
        ws.append(np.stack([W0K, W1K, W0k1, W1k1]))
    return np.stack(ws)  # [3, 4, 128, 128] fp16


_CACHE = {}


def _patch_act_tables():
    """Force every activation onto the natural_log_exp_and_others table set.

    All ACT funcs this kernel uses (Copy, Ln, Exp, Square, Identity) live in
    that one set; by default the set-chooser pass picks different sets per
    func, inserting ~1.3us ACT_TABLE_LOADs between almost every pair of
    activations (~120us/core wasted).  We keep the dict ORDER intact (the
    index is the act_func_set_id walrus resolves against act_info.json) and
    just empty out every other set so the chooser has one valid option.
    """
    import concourse.bacc as bacc_mod
    import concourse.hw_specs as hw_specs
    if getattr(bacc_mod.get_activation_tables, '_mgf_patched', False):
        return
    orig = hw_specs.get_activation_tables
    KEEP = 'natural_log_exp_and_others'

    def patched(arch):
        t = orig(arch)
        return {name: (fns if name == KEEP else set())
                for name, fns in t.items()}
    patched._mgf_patched = True
    bacc_mod.get_activation_tables = patched


def _register_sqdiff():
    """Register a fused out=(in0-in1)^2 custom DVE op (saves one full
    elementwise pass per gradient direction per tile)."""
    import numpy as np
    from concourse import dve_ops
    from concourse.dve_spec import Spec, Src0, Src1, sq, lower, _has_src1
    from concourse.dve_uop import DveOpSpec
    for op in dve_ops.OPS:
        if op.name == "SQDIFF_ANT":
            return op
    spec = Spec(
        body=sq(Src0 - Src1),
        reference=lambda in0, in1, s0, s1, imm2:
            ((np.asarray(in0) - np.asarray(in1)) ** 2).astype(np.float32),
    )
    row = dve_ops._CUSTOM_DVE_ROW_BASE + len(dve_ops.OPS)
    assert row < 0x20
    shas = {}
    for ver in ("v3", "v4"):
        try:
            tmp = DveOpSpec(name="SQDIFF_ANT", opcode=row,
                            uops=lower(spec, ver=ver),
                            rd1_en=_has_src1(spec))
            shas[ver] = tmp.sha(ver)
        except Exception:
            pass
    op = dve_ops.DveOp("SQDIFF_ANT", spec, subdim=False, uops_sha=shas)
    dve_ops.OPS.append(op)
    dve_ops.CUSTOM_DVE_SPECS[op.name] = spec
    dve_ops._SUB_OPCODE_FOR_NAME[op.name] = row
    return op


def build(trace_sim=False):
    """Build + compile the per-core Bass kernel. Returns the Bacc module."""
    if 'nc' in _CACHE:
        return _CACHE['nc']
    import concourse.bacc as bacc
    import concourse.tile as tile
    from concourse import mybir
    from contextlib import ExitStack

    _patch_act_tables()
    SQDIFF = _register_sqdiff()

    f32 = mybir.dt.float32
    f16 = mybir.dt.float16
    ADD = mybir.AluOpType.add
    SUB = mybir.AluOpType.subtract
    AXX = mybir.AxisListType.X
    LN = mybir.ActivationFunctionType.Ln
    EXP = mybir.ActivationFunctionType.Exp

    nc = bacc.Bacc("TRN2", target_bir_lowering=False, debug=False,
                   num_devices=N_CORES)
    xp_d = nc.dram_tensor("xp", [IMG, HP, HP], f16, kind="ExternalInput").ap()
    w_d = nc.dram_tensor("wts", [3, 4, 128, 128], f16, kind="ExternalInput").ap()
    y_d = nc.dram_tensor("y", [IMG, 3, H, W], f32, kind="ExternalOutput").ap()

    with tile.TileContext(nc, trace_sim=trace_sim) as tc, ExitStack() as ctx:
        wp = ctx.enter_context(tc.tile_pool(name="wp", bufs=12))
        xpool = ctx.enter_context(tc.tile_pool(name="xpool", bufs=10))
        pspool = ctx.enter_context(
            tc.tile_pool(name="ps", bufs=8, space="PSUM"))
        ckp = ctx.enter_context(tc.tile_pool(name="ckp", bufs=6))
        c1p = ctx.enter_context(tc.tile_pool(name="c1p", bufs=7))
        s2p = ctx.enter_context(tc.tile_pool(name="s2p", bufs=6))
        ddp = ctx.enter_context(tc.tile_pool(name="ddp", bufs=7))
        gxp = ctx.enter_context(tc.tile_pool(name="gxp", bufs=6))
        gyp = ctx.enter_context(tc.tile_pool(name="gyp", bufs=4))
        shp = ctx.enter_context(tc.tile_pool(name="shp", bufs=4))
        resp = ctx.enter_context(tc.tile_pool(name="resp", bufs=4))
        inip = ctx.enter_context(tc.tile_pool(name="inip", bufs=16))

        # stationary weights, loaded once
        wt = {}
        for s in range(3):
            for j in range(4):
                t = wp.tile([128, 128], f16, tag="w")
                nc.sync.dma_start(t[:], w_d[s, j])
                wt[(s, j)] = t

        for img in range(IMG):
            # load padded image rows into 5 aligned tiles
            xt = []
            for m in range(5):
                t = xpool.tile([ROWS[m], HP], f16, tag="xp")
                nc.sync.dma_start(t[:], xp_d[img, 128 * m: 128 * m + ROWS[m], :])
                xt.append(t)

            for s, k in enumerate(KS):
                c0 = PAD - k - 1            # xp col index of v = -k-1
                ncol = 513 + 2 * k          # CK/Ck1 cols: v in [-k-1, 511+k]
                nb = ncol - 512             # second psum chunk (2k+1)
                wb = PAD + k                # rows of W1 slice that matter (<=35)

                # ---------------- gx chain: CK = vbox_K(xp) ----------------
                gx_tiles = []
                for t in range(4):
                    psA = pspool.tile([128, 512], f32, tag="ps")
                    psB = pspool.tile([128, nb], f32, tag="ps")
                    w0, w1 = wt[(s, 0)], wt[(s, 1)]
                    nc.tensor.matmul(psA[:], w0[:], xt[t][:, c0:c0 + 512],
                                     start=True, stop=False)
                    nc.tensor.matmul(psB[:], w0[:], xt[t][:, c0 + 512:c0 + ncol],
                                     start=True, stop=False)
                    nc.tensor.matmul(psA[:], w1[0:wb, :],
                                     xt[t + 1][0:wb, c0:c0 + 512],
                                     start=False, stop=True)
                    nc.tensor.matmul(psB[:], w1[0:wb, :],
                                     xt[t + 1][0:wb, c0 + 512:c0 + ncol],
                                     start=False, stop=True)
                    cks = ckp.tile([128, ncol], f32, tag="ck")
                    nc.scalar.copy(cks[:, 0:512], psA[:])
                    nc.scalar.copy(cks[:, 512:ncol], psB[:])
                    # S2[y][x'] = sum_{v=x'-k..x'} CK[y][v],  x' in [0, 511+k]
                    ini = inip.tile([128, 1], f32, tag="ini")
                    nc.vector.reduce_sum(ini[:], cks[:, 0:k + 1], axis=AXX)
                    s2 = s2p.tile([128, 512 + k], f32, tag="s2")
                    nc.vector.tensor_tensor_scan(
                        s2[:], cks[:, k + 1:k + 1 + 512 + k],
                        cks[:, 0:512 + k], ini[:], ADD, SUB)
                    nc.scalar.activation(s2[:], s2[:], LN)
                    # gx^2 = (lnS2[x] - lnS2[x+k])^2 fused in one DVE pass
                    gx = gxp.tile([128, 512], f32, tag="gx")
                    nc.vector._custom_dve(SQDIFF, out=gx[:],
                                          in0=s2[:, 0:512], in1=s2[:, k:k + 512])
                    gx_tiles.append(gx)

                # ---------------- gy chain: Ck1 = vbox_{k+1}(xp) -----------
                # out array row j = y' + k in [0, 528]; 5 tiles (last 17 rows)
                lnd = []
                for t in range(5):
                    ro = 128 if t < 4 else 17
                    psA = pspool.tile([128, 512], f32, tag="ps")
                    psB = pspool.tile([128, nb], f32, tag="ps")
                    w2, w3 = wt[(s, 2)], wt[(s, 3)]
                    if t < 4:
                        nc.tensor.matmul(psA[:], w2[:], xt[t][:, c0:c0 + 512],
                                         start=True, stop=False)
                        nc.tensor.matmul(psB[:], w2[:],
                                         xt[t][:, c0 + 512:c0 + ncol],
                                         start=True, stop=False)
                        nc.tensor.matmul(psA[:], w3[0:wb, :],
                                         xt[t + 1][0:wb, c0:c0 + 512],
                                         start=False, stop=True)
                        nc.tensor.matmul(psB[:], w3[0:wb, :],
                                         xt[t + 1][0:wb, c0 + 512:c0 + ncol],
                                         start=False, stop=True)
                    else:
                        nc.tensor.matmul(psA[0:ro, :], w2[0:ROWS[4], 0:ro],
                                         xt[4][:, c0:c0 + 512],
                                         start=True, stop=True)
                        nc.tensor.matmul(psB[0:ro, :], w2[0:ROWS[4], 0:ro],
                                         xt[4][:, c0 + 512:c0 + ncol],
                                         start=True, stop=True)
                    c1 = c1p.tile([128, ncol], f32, tag="c1")
                    nc.scalar.copy(c1[0:ro, 0:512], psA[0:ro, :])
                    nc.scalar.copy(c1[0:ro, 512:ncol], psB[0:ro, :])
                    # D[j][x] = sum_{v=x-k..x+k} Ck1[j][v], x in [0, 511]
                    ini = inip.tile([128, 1], f32, tag="ini")
                    nc.vector.reduce_sum(ini[0:ro, :], c1[0:ro, 0:2 * k + 1],
                                         axis=AXX)
                    dd = ddp.tile([128, 512], f32, tag="dd")
                    nc.vector.tensor_tensor_scan(
                        dd[0:ro, :], c1[0:ro, 2 * k + 1:2 * k + 1 + 512],
                        c1[0:ro, 0:512], ini[0:ro, :], ADD, SUB)
                    nc.scalar.activation(dd[0:ro, :], dd[0:ro, :], LN)
                    lnd.append(dd)

                # ------------- combine: gy, square-sum, sqrt, store --------
                for t in range(4):
                    sh = shp.tile([128, 512], f32, tag="sh")
                    nc.gpsimd.dma_start(sh[0:128 - k, :], lnd[t][k:128, :])
                    nc.gpsimd.dma_start(sh[128 - k:128, :], lnd[t + 1][0:k, :])
                    # gy^2 = (lnD[y] - lnD[y+k])^2 fused in one DVE pass
                    gy = gyp.tile([128, 512], f32, tag="gy")
                    nc.vector._custom_dve(SQDIFF, out=gy[:],
                                          in0=lnd[t][:], in1=sh[:])
                    gx = gx_tiles[t]
                    # s = gx^2 + gy^2 + tiny (tiny keeps ln finite when s==0)
                    nc.vector.scalar_tensor_tensor(gx[:], gx[:], 1e-38, gy[:],
                                                   ADD, ADD)
                    # sqrt(s) = exp(0.5*ln(s)): keeps every ACT op in the
                    # natural_log_exp_and_others table set (no ~2.7us swaps)
                    nc.scalar.activation(gx[:], gx[:], LN)
                    res = resp.tile([128, 512], f32, tag="res")
                    nc.scalar.activation(res[:], gx[:], EXP, scale=0.5)
                    nc.sync.dma_start(y_d[img, s, 128 * t:128 * t + 128, :],
                                      res[:])

    nc.compile()
    _CACHE['nc'] = nc
    return nc


def prep_inputs(x):
    """x: [32,1,512,512] f32 -> per-core input maps."""
    x2 = np.asarray(x, dtype=np.float32).reshape(32, 512, 512)
    xp = np.pad(x2, ((0, 0), (PAD, PAD), (PAD, PAD)), mode='reflect') + 0.01
    xp16 = xp.astype(np.float16)
    wts = _make_weights()
    return [{"xp": np.ascontiguousarray(xp16[c * IMG:(c + 1) * IMG]),
             "wts": wts} for c in range(N_CORES)]


def run(x, trace=False, **kw):
    from concourse.bass_utils import run_bass_kernel_spmd
    nc = build()
    in_maps = prep_inputs(x)
    res = run_bass_kernel_spmd(nc, in_maps, list(range(N_CORES)),
                               trace=trace, **kw)
    out = np.concatenate([res.results[c]["y"] for c in range(N_CORES)], axis=0)
    return out.astype(np.float32), res


def kernel(x):
    out, _ = run(x, trace=False)
    return out
